# revision 62
# baseline (speedup 1.0000x reference)
"""Multi-head attention (RoPE, causal) Trainium2 Bass kernel, 8-way sharded.

Sharding: tensor-parallel over heads x data-parallel over batch.
  core c (0..7): batch b = c // 4, head group hg = c % 4 -> heads [4*hg, 4*hg+4).
Each core computes its 4 heads' QKV projection, RoPE, causal attention, and a
partial output projection (its 512 columns of the E-dim contraction).  The host
sums the 4 partials per batch and transposes back.

Device-side layouts are transposed ([feature, seq]) so matmuls feed the PE
array directly.  All matmul operands are bf16 (full PE rate at any tile width;
fp32 PSUM accumulation), which keeps every tensor SBUF-resident — no DRAM
q/k spill.  Softmax skips the max-subtraction (logits are O(+-10) so fp32 exp
cannot overflow; bf16 ex stores magnitudes up to e^+big fine) and stays in the
[key, query] orientation; the denominator is accumulated on the Vector engine
and partition-reduced on GPSIMD, so the Tensor engine runs only real FLOPs.

The whole kernel is one fused software-pipelined loop over 512-row q-blocks:
the QKV chains for block sb+1 and the output projection for block sb-1 are
emitted interleaved ("fillers") into block sb's attention tiles, so the
in-order Tensor queue always has independent matmuls to run while the
Activation engine works through the exp() pipeline.
"""

import sys

sys.path.insert(0, "/opt/trn_rl_repo")

import numpy as np
import ml_dtypes

import concourse.bass as bass  # noqa: F401
import concourse.tile as tile
from concourse import bacc, mybir
from concourse import bass_utils

# bass_utils' trace path imports antenv.axon_hooks, which may be absent from
# this image; register a no-op hook module so an externally-set BASS_TRACE
# degrades to "no profile" instead of crashing the run.
try:
    import antenv.axon_hooks  # noqa: F401
except ImportError:
    import types

    _hooks = types.ModuleType("antenv.axon_hooks")
    _hooks.get_axon_ntff_profile_hook = lambda: None
    _hooks.set_axon_ntff_profile_hook = lambda h: None
    sys.modules["antenv.axon_hooks"] = _hooks
    try:
        import antenv

        antenv.axon_hooks = _hooks
    except ImportError:
        pass

# Problem shape (hardcoded per contract).
B = 2
S = 2048
E = 2048
H = 16
D = 128
N_CORES = 8
GPB = N_CORES // B  # head groups per batch = 4
HPC = H // GPB  # heads per core = 4
DPC = HPC * D  # feature cols per core = 512
SBLK = 512
NSBLK = S // SBLK  # 4
NECH = E // 128  # 16 contraction chunks
SM_SCALE = float(D) ** -0.5

F32 = mybir.dt.float32
BF16 = mybir.dt.bfloat16

_CACHE = {}
_RUN_KWARGS = {}


def _build_nc():
    nc = bacc.Bacc(
        "TRN2",
        target_bir_lowering=False,
        debug=False,
        enable_asserts=True,
        num_devices=N_CORES,
    )
    xT = nc.dram_tensor("xT", [E, S], BF16, kind="ExternalInput").ap()
    wqT = nc.dram_tensor("wqT", [E, DPC], BF16, kind="ExternalInput").ap()
    wkT = nc.dram_tensor("wkT", [E, DPC], BF16, kind="ExternalInput").ap()
    wvT = nc.dram_tensor("wvT", [E, DPC], BF16, kind="ExternalInput").ap()
    woutT = nc.dram_tensor("woutT", [DPC, E], BF16, kind="ExternalInput").ap()
    cosT = nc.dram_tensor("cosT", [D, S], BF16, kind="ExternalInput").ap()
    sinTs = nc.dram_tensor("sinTs", [D, S], BF16, kind="ExternalInput").ap()
    bmask = nc.dram_tensor("bmask", [128, 4 * SBLK], BF16, kind="ExternalInput").ap()
    outT = nc.dram_tensor("outT", [E, S], BF16, kind="ExternalOutput").ap()

    with tile.TileContext(nc) as tc, nc.allow_low_precision(reason="bf16 matmuls"):
        with (
            tc.tile_pool(name="persist", bufs=1) as persist,
            tc.tile_pool(name="wq", bufs=NECH) as wq_pool,
            tc.tile_pool(name="wk", bufs=NECH) as wk_pool,
            tc.tile_pool(name="wv", bufs=NECH) as wv_pool,
            tc.tile_pool(name="wo", bufs=HPC) as wo_pool,
            tc.tile_pool(name="xts", bufs=28) as xts_pool,
            tc.tile_pool(name="rt1", bufs=2) as t1_pool,
            tc.tile_pool(name="rt2", bufs=2) as t2_pool,
            tc.tile_pool(name="ex", bufs=9) as ex_pool,
            tc.tile_pool(name="acc", bufs=2) as acc_pool,
            tc.tile_pool(name="accb", bufs=1) as accb_pool,
            tc.tile_pool(name="rcpr", bufs=2) as rcpr_pool,
            tc.tile_pool(name="dbc", bufs=2) as dbc_pool,
            tc.tile_pool(name="ot", bufs=5) as ot_pool,
            tc.tile_pool(name="pmm", bufs=3, space="PSUM") as pmm_pool,
            tc.tile_pool(name="psc", bufs=3, space="PSUM") as psc_pool,
            tc.tile_pool(name="pctx", bufs=2, space="PSUM") as pctx_pool,
        ):
            ones_bf = persist.tile([128, 1], BF16, tag="ones", name="ones_bf")
            nc.vector.memset(ones_bf[:], 1.0)
            cos_sb = persist.tile([D, S], BF16, tag="cos", name="cos_sb")
            sin_sb = persist.tile([D, S], BF16, tag="sin", name="sin_sb")
            bm_sb = persist.tile([128, 4 * SBLK], BF16, tag="bm", name="bm_sb")
            q_sb = [
                persist.tile([128, S], BF16, tag=f"q{h}", name=f"q{h}")
                for h in range(HPC)
            ]
            k_sb = [
                persist.tile([128, S], BF16, tag=f"k{h}", name=f"k{h}")
                for h in range(HPC)
            ]
            v_sb = persist.tile([128, 4 * NSBLK * DPC], BF16, tag="v", name="v_sb")
            ctx16 = [
                [
                    persist.tile([128, SBLK], BF16, tag=f"c{h}_{t}", name=f"c{h}_{t}")
                    for t in range(NSBLK)
                ]
                for h in range(HPC)
            ]

            # Input loads alternate between the sync and gpsimd queues: one
            # queue tops out ~185GB/s, and a DMA trigger costs ~600ns on the
            # issuing engine, so only idle-ish engines may carry triggers
            # (scalar/vector triggers poison the attention pipeline).
            engq = [nc.sync]
            qi = [0]

            def dma(out, in_):
                engq[qi[0] % len(engq)].dma_start(out, in_)
                qi[0] += 1

            # weights + first x block, interleaved so the first q chain's
            # dependencies land earliest; rope/mask tables slot in after the
            # first few chunks (they are not needed until the first rope)
            wq_t, wk_t, wv_t, wo_t = [], [], [], []
            xts0 = []
            for e in range(NECH):
                wt = wq_pool.tile([128, DPC], BF16, tag="wq", name="wq")
                dma(wt[:], wqT[e * 128 : (e + 1) * 128, :])
                wq_t.append(wt)
                xt = xts_pool.tile([128, SBLK], BF16, tag="xt", name="xt")
                dma(xt[:], xT[e * 128 : (e + 1) * 128, 0:SBLK])
                xts0.append(xt)
                if e == 7:
                    dma(cos_sb[:], cosT[:])
                    dma(sin_sb[:], sinTs[:])
            for e in range(NECH):
                wt = wk_pool.tile([128, DPC], BF16, tag="wk", name="wk")
                dma(wt[:], wkT[e * 128 : (e + 1) * 128, :])
                wk_t.append(wt)
            for e in range(NECH):
                wt = wv_pool.tile([128, DPC], BF16, tag="wv", name="wv")
                dma(wt[:], wvT[e * 128 : (e + 1) * 128, :])
                wv_t.append(wt)
            dma(bm_sb[:], bmask[:])
            for h in range(HPC):
                wt = wo_pool.tile([128, E], BF16, tag="wo", name="wo")
                dma(wt[:], woutT[h * 128 : (h + 1) * 128, :])
                wo_t.append(wt)

            def rope_to(ps, dst, ssl):
                # dst[:, ssl] = ps*cos + rotate_half(ps)*sin  (sin pre-negated
                # in its first half by the host table)
                t1 = t1_pool.tile([128, SBLK], F32, tag="t1", name="t1")
                nc.vector.tensor_mul(t1[:], ps[:], cos_sb[:, ssl])
                t2 = t2_pool.tile([128, SBLK], F32, tag="t2", name="t2")
                nc.vector.tensor_mul(t2[0:64, :], ps[64:128, :], sin_sb[0:64, ssl])
                nc.vector.tensor_mul(t2[64:128, :], ps[0:64, :], sin_sb[64:128, ssl])
                nc.vector.tensor_add(dst[:, ssl], t1[:], t2[:])

            def emit_xloads(sb):
                # mid-kernel loads stay on sync: a gpsimd trigger's WAR wait
                # would head-of-line-block the broadcasts behind it
                xts = []
                for e in range(NECH):
                    xt = xts_pool.tile([128, SBLK], BF16, tag="xt", name="xt")
                    nc.sync.dma_start(
                        xt[:], xT[e * 128 : (e + 1) * 128, sb * SBLK : (sb + 1) * SBLK]
                    )
                    xts.append(xt)
                return xts

            def gen_qkv(sb, xts):
                # yields once per Tensor matmul (192 total)
                ssl = slice(sb * SBLK, (sb + 1) * SBLK)
                for m in range(HPC):
                    for w_t, dst in ((wq_t, q_sb), (wk_t, k_sb)):
                        ps = pmm_pool.tile([128, SBLK], F32, tag="mm", name="psqk")
                        for e in range(NECH):
                            nc.tensor.matmul(
                                ps[:],
                                w_t[e][:, m * 128 : (m + 1) * 128],
                                xts[e][:],
                                start=(e == 0),
                                stop=(e == NECH - 1),
                                skip_group_check=True,
                            )
                            if e == NECH - 1:
                                rope_to(ps, dst[m], ssl)
                            yield
                for sm in range(SBLK // 128):
                    st = sb * (SBLK // 128) + sm
                    ps = pmm_pool.tile([128, DPC], F32, tag="mm", name="psv")
                    for e in range(NECH):
                        nc.tensor.matmul(
                            ps[:],
                            xts[e][:, sm * 128 : (sm + 1) * 128],
                            wv_t[e][:],
                            start=(e == 0),
                            stop=(e == NECH - 1),
                            skip_group_check=True,
                        )
                        if e == NECH - 1:
                            nc.scalar.copy(v_sb[:, st * DPC : (st + 1) * DPC], ps[:])
                        yield

            def gen_outproj(sb, copy_alt=False, store_gp=False):
                # yields once per Tensor matmul (64 total); copy_alt splits the
                # PSUM-drain copies between Act and DVE (GPSIMD cannot read
                # PSUM) when Act is the pipeline limiter in the enclosing
                # attention block; store_gp additionally alternates the store
                # queue onto gpsimd (safe only when no broadcasts remain)
                ssl = slice(sb * SBLK, (sb + 1) * SBLK)
                for m in range(E // 128):
                    po = pmm_pool.tile([128, SBLK], F32, tag="mm", name="po")
                    for h in range(HPC):
                        nc.tensor.matmul(
                            po[:],
                            wo_t[h][:, m * 128 : (m + 1) * 128],
                            ctx16[h][sb][:],
                            start=(h == 0),
                            stop=(h == HPC - 1),
                            skip_group_check=True,
                        )
                        if h == HPC - 1:
                            ot = ot_pool.tile([128, SBLK], BF16, tag="ot", name="ot")
                            if copy_alt and m % 2:
                                nc.vector.tensor_scalar_mul(ot[:], po[:], 1.0)
                            else:
                                nc.scalar.copy(ot[:], po[:])
                            st_eng = nc.gpsimd if (store_gp and m % 2) else nc.sync
                            st_eng.dma_start(outT[m * 128 : (m + 1) * 128, ssl], ot[:])
                        yield

            def chain_caps(seq):
                for g, cap in seq:
                    for _ in range(cap):
                        try:
                            next(g)
                        except StopIteration:
                            break

            def attn(sb, filler_iter, total_steps):
                njt = 4 * (sb + 1)
                n_tiles = HPC * njt
                tile_i = 0
                drawn = 0
                for h in range(HPC):
                    # The very last head's softmax tail gates the epilogue
                    # behind the deep end-of-kernel DVE queue; for it alone,
                    # accumulate the denominator with per-tile ones-matmuls on
                    # the (then slack) Tensor engine so the tail is ~2us.
                    den_on_pe = sb == NSBLK - 1 and h == HPC - 1
                    if not den_on_pe:
                        acc = acc_pool.tile([128, SBLK], F32, tag="acc", name="acc")
                    ctx_ps = pctx_pool.tile([128, SBLK], F32, tag="ctx", name="ctxps")
                    if den_on_pe:
                        den_ps = pctx_pool.tile([1, SBLK], F32, tag="ctx", name="denps2")

                    def emit_ctx(work):
                        jt, lo, ex = work
                        if den_on_pe:
                            nc.tensor.matmul(
                                den_ps[0:1, lo:SBLK],
                                ones_bf[:],
                                ex[:, lo:SBLK],
                                start=(jt == 0),
                                stop=(jt == njt - 1),
                                skip_group_check=True,
                            )
                        nc.tensor.matmul(
                            ctx_ps[:, lo:SBLK],
                            v_sb[:, jt * DPC + h * 128 : jt * DPC + (h + 1) * 128],
                            ex[:, lo:SBLK],
                            start=(jt == 0),
                            stop=(jt == njt - 1),
                            skip_group_check=True,
                        )

                    inflight = []
                    for jt in range(njt):
                        o = jt - 4 * sb
                        # causal: columns i < jt*128 of this q-block are fully
                        # masked for this j-tile -> shrink width
                        lo = max(o, 0) * 128
                        sc = psc_pool.tile([128, SBLK], F32, name="sc")
                        nc.tensor.matmul(
                            sc[:, lo:SBLK],
                            k_sb[h][:, jt * 128 : (jt + 1) * 128],
                            q_sb[h][:, sb * SBLK + lo : (sb + 1) * SBLK],
                            start=True,
                            stop=True,
                            skip_group_check=True,
                        )
                        ex = ex_pool.tile([128, SBLK], BF16, tag="ex", name="ex")
                        nc.scalar.activation(
                            ex[:, lo:SBLK],
                            sc[:, lo:SBLK],
                            mybir.ActivationFunctionType.Exp,
                            scale=SM_SCALE,
                        )
                        if o >= 0:
                            nc.vector.tensor_mul(
                                ex[:, lo:SBLK],
                                ex[:, lo:SBLK],
                                bm_sb[:, o * SBLK + lo : (o + 1) * SBLK],
                            )
                        if not den_on_pe:
                            if jt == 0:
                                nc.vector.tensor_scalar_mul(acc[:], ex[:], 1.0)
                            else:
                                nc.vector.tensor_add(
                                    acc[:, lo:SBLK], acc[:, lo:SBLK], ex[:, lo:SBLK]
                                )
                        inflight.append((jt, lo, ex))
                        if len(inflight) > 6:
                            emit_ctx(inflight.pop(0))
                        tile_i += 1
                        want = (total_steps * tile_i) // n_tiles
                        if want > drawn:
                            for _ in range(want - drawn):
                                next(filler_iter, None)
                            drawn = want
                    for work in inflight:
                        emit_ctx(work)
                    if not den_on_pe:
                        # denominator: bf16 copy of acc -> K=128 ones-matmul
                        # partition sum -> [1,512] reciprocal -> gpsimd bcast
                        acc_bf = accb_pool.tile([128, SBLK], BF16, tag="ab", name="ab")
                        nc.vector.tensor_scalar_mul(acc_bf[:], acc[:], 1.0)
                        den_ps = pmm_pool.tile([1, SBLK], F32, tag="mm", name="denps")
                        nc.tensor.matmul(
                            den_ps[:],
                            ones_bf[:],
                            acc_bf[:],
                            start=True,
                            stop=True,
                            skip_group_check=True,
                        )
                    rcp_row = rcpr_pool.tile([1, SBLK], F32, tag="rr", name="rr")
                    nc.vector.reciprocal_approx_fast(out=rcp_row[:], in_=den_ps[:])
                    dbc = dbc_pool.tile([128, SBLK], F32, tag="db", name="db")
                    nc.gpsimd.partition_broadcast(dbc[:], rcp_row[:])
                    nc.vector.tensor_mul(ctx16[h][sb][:], ctx_ps[:], dbc[:])
                for _ in iter(lambda: next(filler_iter, StopIteration), StopIteration):
                    pass

            # ---- prologue: QKV for block 0, no interleave ----
            for _ in gen_qkv(0, xts0):
                pass
            # ---- fused attention loop ----
            for sb in range(NSBLK):
                seq = []
                total = 0
                qg = None
                if sb + 1 < NSBLK:
                    xts_next = emit_xloads(sb + 1)
                    qg = gen_qkv(sb + 1, xts_next)
                    total += 2 * HPC * NECH + (SBLK // 128) * NECH
                if sb >= 1:
                    og = gen_outproj(sb - 1, copy_alt=(sb == 3))
                    total += (E // 128) * HPC
                    if qg is not None:
                        seq = [(qg, 16), (og, 1 << 30), (qg, 1 << 30)]
                    else:
                        seq = [(og, 1 << 30)]
                elif qg is not None:
                    seq = [(qg, 1 << 30)]

                def filler_iter_fn(entries):
                    for g, cap in entries:
                        n = 0
                        while n < cap:
                            try:
                                next(g)
                            except StopIteration:
                                break
                            n += 1
                            yield

                attn(sb, filler_iter_fn(seq), total)
            # ---- epilogue: output projection for the last block ----
            # software-pipelined: each chain's h0..h2 matmuls run ahead while
            # only the h3 final waits on the last softmax tail
            lsb = NSBLK - 1
            ssl3 = slice(lsb * SBLK, (lsb + 1) * SBLK)

            def finish_po(m, po):
                nc.tensor.matmul(
                    po[:],
                    wo_t[HPC - 1][:, m * 128 : (m + 1) * 128],
                    ctx16[HPC - 1][lsb][:],
                    start=False,
                    stop=True,
                    skip_group_check=True,
                )
                ot = ot_pool.tile([128, SBLK], BF16, tag="ot", name="ot")
                nc.scalar.copy(ot[:], po[:])
                nc.sync.dma_start(outT[m * 128 : (m + 1) * 128, ssl3], ot[:])

            pend_po = []
            for m in range(E // 128):
                po = pmm_pool.tile([128, SBLK], F32, tag="mm", name="po")
                for h in range(HPC - 1):
                    nc.tensor.matmul(
                        po[:],
                        wo_t[h][:, m * 128 : (m + 1) * 128],
                        ctx16[h][lsb][:],
                        start=(h == 0),
                        stop=False,
                        skip_group_check=True,
                    )
                pend_po.append((m, po))
                if len(pend_po) > 2:
                    finish_po(*pend_po.pop(0))
            for m, po in pend_po:
                finish_po(m, po)

    nc.compile()
    return nc


def _rope_tables():
    inv_freq = 1.0 / (10000.0 ** (np.arange(0, D, 2, dtype=np.float64) / D))
    t = np.arange(S, dtype=np.float64)
    freqs = np.outer(t, inv_freq)  # (S, D/2)
    emb = np.concatenate([freqs, freqs], axis=-1)  # (S, D)
    cosT = np.cos(emb).T.astype(np.float32).copy()  # (D, S)
    sinT = np.sin(emb).T.astype(np.float32)
    sinTs = sinT.copy()
    sinTs[: D // 2] = -sinT[: D // 2]
    return cosT, np.ascontiguousarray(sinTs)


def _binmask():
    r = np.arange(128)[:, None]
    c = np.arange(SBLK)[None, :]
    blocks = [(r + o * 128 <= c).astype(np.float32) for o in range(4)]
    return np.ascontiguousarray(np.concatenate(blocks, axis=1))


def _bf16(a):
    return np.ascontiguousarray(np.asarray(a, dtype=ml_dtypes.bfloat16))





def _numpy_fallback(x, mask, wqkv, bqkv, wout, bout):
    qkv = x @ wqkv.T + bqkv
    q, k, v = np.split(qkv, 3, axis=-1)
    q = q.reshape(B, S, H, D).transpose(0, 2, 1, 3)
    k = k.reshape(B, S, H, D).transpose(0, 2, 1, 3)
    v = v.reshape(B, S, H, D).transpose(0, 2, 1, 3)
    inv_freq = 1.0 / (10000.0 ** (np.arange(0, D, 2, dtype=np.float32) / D))
    t = np.arange(S, dtype=np.float32)
    freqs = np.outer(t, inv_freq)
    emb = np.concatenate([freqs, freqs], axis=-1)
    cos, sin = np.cos(emb), np.sin(emb)

    def rot(a):
        a1, a2 = np.split(a, 2, axis=-1)
        return np.concatenate([-a2, a1], axis=-1)

    q = q * cos + rot(q) * sin
    k = k * cos + rot(k) * sin
    scores = np.einsum("bhqd,bhkd->bhqk", q, k) * SM_SCALE
    scores = np.where(mask, -np.inf, scores)
    scores = scores - scores.max(axis=-1, keepdims=True)
    w = np.exp(scores)
    w = w / w.sum(axis=-1, keepdims=True)
    ctx = np.einsum("bhqk,bhkd->bhqd", w, v)
    ctx = ctx.transpose(0, 2, 1, 3).reshape(B, S, E)
    return (ctx @ wout.T + bout).astype(np.float32)


def kernel(x, mask, wqkv, bqkv, wout, bout, **_):
    x = np.ascontiguousarray(np.asarray(x), dtype=np.float32)
    wqkv = np.ascontiguousarray(np.asarray(wqkv), dtype=np.float32)
    bqkv = np.asarray(bqkv, dtype=np.float32)
    wout = np.ascontiguousarray(np.asarray(wout), dtype=np.float32)
    bout = np.asarray(bout, dtype=np.float32)
    mask = np.asarray(mask)

    causal = np.array_equal(mask, np.triu(np.ones((S, S), dtype=bool), k=1))
    if not causal or np.any(bqkv):
        return _numpy_fallback(x, mask, wqkv, bqkv, wout, bout)

    if "nc" not in _CACHE:
        _CACHE["nc"] = _build_nc()
    nc = _CACHE["nc"]

    cosT, sinTs = _rope_tables()
    bm = _binmask()

    in_maps = []
    for c in range(N_CORES):
        b, hg = divmod(c, GPB)
        cols = slice(hg * DPC, (hg + 1) * DPC)
        wq = wqkv[0 * E : 1 * E, :][cols, :]  # (512, E)
        wk = wqkv[1 * E : 2 * E, :][cols, :]
        wv = wqkv[2 * E : 3 * E, :][cols, :]
        in_maps.append(
            {
                "xT": _bf16(x[b].T),
                "wqT": _bf16(wq.T),
                "wkT": _bf16(wk.T),
                "wvT": _bf16(wv.T),
                "woutT": _bf16(wout[:, cols].T),  # (512, E)
                "cosT": _bf16(cosT),
                "sinTs": _bf16(sinTs),
                "bmask": _bf16(bm),
            }
        )

    res = bass_utils.run_bass_kernel_spmd(
        nc, in_maps, core_ids=list(range(N_CORES)), **_RUN_KWARGS
    )
    _CACHE["last_results"] = res

    out = np.empty((B, S, E), dtype=np.float32)
    for b in range(B):
        acc = res.results[b * GPB]["outT"].astype(np.float32)
        for g in range(1, GPB):
            acc += res.results[b * GPB + g]["outT"].astype(np.float32)
        out[b] = acc.T
    out += bout
    return out
